# revision 10
# baseline (speedup 1.0000x reference)
"""Chamfer distance on 8 Trainium2 NeuronCores.

Problem: x1 (8, 4096, 3) f32, y1 (8, 4096, 3) f32.
  d2[b,m,n] = |y[b,m] - x[b,n]|^2
  out = mean_{b,n}(min_m sqrt(1e-8 + max(d2,0))) + mean_{b,m}(min_n ...)

Strategy (data-parallel over B, one batch element per core):
  * sqrt / +eps / max(.,0) are monotonic -> compute mins over raw d2 and
    apply them only to the reduced 4096-vectors on the host.
  * -d2 is produced in PSUM by a single matmul with augmented K=30 inputs
    (3-level bf16 split of each fp32 operand, ~2^-26 accurate); the y side
    is negated so all on-device mins become maxes.
  * the PE runs TWO row-tiled streams (tile rows 0 and 2 of the 32x128
    tiling grid, operands replicated at partition bases 0 and 64) so the
    two weight streams overlap and LDWEIGHTS hides under matmuls.
  * per m-tile [128, 4096] f32 PSUM:
      - ScalarE casts the two 2048-col chunks to a bf16 SBUF slab (the
        PSUM-evacuation cost, ~1.9us/chunk, is the pipeline governor);
        a few chunks are cast by the DVE instead to balance load.
      - direction A (min over n per m) is ONE fused DVE op:
        tensor_tensor_reduce(max, max) over the two slab halves writes
        the per-partition running max straight into macc[:, mt].
      - direction B (min over m per n) is an elementwise running max of
        the slabs, split into TWO independent chains: most tiles on the
        DVE (bf16 2x), a stride of tiles on the otherwise-idle GPSIMD;
        the chains merge at the end.
  * outputs: macc [128, 32] f32 (dirA, complete) and the merged dirB
    accumulator [128, 4096] bf16 whose 128-partition max is done on the
    host.  Output DMA is split across 4 queues to shorten the tail.
"""

import os
import sys

for _p in ("/opt/trn_rl_repo", "/root/.axon_site/_ro/trn_rl_repo"):
    if os.path.isdir(_p) and _p not in sys.path:
        sys.path.insert(0, _p)
        break

import numpy as np
import ml_dtypes

_B = 8
_N = 4096          # points per cloud (both x and y)
_K = 30            # augmented contraction dim (3-level bf16 split)
_NCORES = 8
_MT = _N // 128    # 32 m-tiles
_CHUNK = 2048      # PSUM chunk (4 banks); 2 chunks per m-tile

_BF16 = ml_dtypes.bfloat16

# ---- tuning knobs ----
# m-tiles whose dirB running-max runs on GPSIMD (independent chain).
_GP_TILES = frozenset(int(t) for t in
                      os.environ.get("CH_GP_TILES", "2,4,6,8,10,12,14,16,18,20,22,24").split(",") if t != "")
# (mt, chunk) casts done by the DVE instead of ScalarE.
_DVE_CASTS = frozenset((int(t), 1) for t in
                       os.environ.get("CH_DVE_CAST_TILES", "15,27").split(",") if t != "")
# bisect knobs
_STREAMS = int(os.environ.get("CH_STREAMS", "2"))     # 1 or 2 PE tile rows
_DIRA = os.environ.get("CH_DIRA", "ttr")              # "ttr" | "reduce"

_PROGRAM = None


def _build_program():
    import concourse.bacc as bacc
    import concourse.tile as tile
    import concourse.mybir as mybir

    f32 = mybir.dt.float32
    bf16 = mybir.dt.bfloat16
    MAX = mybir.AluOpType.max

    nc = bacc.Bacc("TRN2", target_bir_lowering=False, debug=False,
                   num_devices=_NCORES)

    xh_d = nc.dram_tensor("xh", [_K, _N], bf16, kind="ExternalInput")
    yh_d = nc.dram_tensor("yh", [_K, _N], bf16, kind="ExternalInput")
    outm_d = nc.dram_tensor("outm", [128, _MT], f32, kind="ExternalOutput")
    outb_d = nc.dram_tensor("outb", [128, _N], bf16, kind="ExternalOutput")

    with tile.TileContext(nc) as tc:
        with tc.tile_pool(name="singles", bufs=1) as singles:
            # operand replicas at partition bases 0 and 64 (PE tile rows
            # 0 and 2 of the 32x128 row-tiling grid)
            xh_s = singles.tile([128, _N], bf16)
            yh_s = singles.tile([128, _N], bf16)
            macc = singles.tile([128, _MT], f32)
            accD = singles.tile([128, _N], bf16)
            accG = (singles.tile([128, _N], bf16, name="accG")
                    if _GP_TILES else None)

            # input DMAs: 3 queues (sync/scalar/gpsimd are the only DMA
            # engines).  Small leading pieces let the first matmuls start
            # early; each piece is needed at a known pipeline time, so
            # order per queue accordingly.  Streams r=0/1 read partition
            # bases 0/64.
            for base, qa in ((0, nc.sync), (64, nc.scalar)):
                qa.dma_start(out=yh_s[base:base + _K, 0:256],
                             in_=yh_d.ap()[:, 0:256])
                qa.dma_start(out=xh_s[base:base + _K, 0:512],
                             in_=xh_d.ap()[:, 0:512])
                qa.dma_start(out=xh_s[base:base + _K, 512:2048],
                             in_=xh_d.ap()[:, 512:2048])
            # xh chunk-1 halves (needed ~5us in) on the gpsimd queue
            for base in (0, 64):
                nc.gpsimd.dma_start(out=xh_s[base:base + _K, 2048:_N],
                                    in_=xh_d.ap()[:, 2048:_N])
            # yh tails (needed progressively, tile k at ~k*1.9us)
            for base, qa in ((0, nc.sync), (64, nc.scalar)):
                qa.dma_start(out=yh_s[base:base + _K, 256:2048],
                             in_=yh_d.ap()[:, 256:2048])
                qa.dma_start(out=yh_s[base:base + _K, 2048:_N],
                             in_=yh_d.ap()[:, 2048:_N])

            with tc.tile_pool(name="psum0", bufs=1, space="PSUM") as psum0, \
                 tc.tile_pool(name="psum1", bufs=1, space="PSUM") as psum1, \
                 tc.tile_pool(name="castp", bufs=6) as castp, \
                 tc.tile_pool(name="ttrout", bufs=2) as ttrout:
                psum_pools = (psum0, psum1)
                first_dve = []   # deferred slabs for accD init
                first_gp = [True]

                for p in range(_MT // 2):
                    slabs = (castp.tile([128, _N], bf16, name="slab0"),
                             castp.tile([128, _N], bf16, name="slab1"))
                    # interleave the two streams chunk-by-chunk so each
                    # stream's PSUM wait hides under the other's matmuls
                    for c in range(2):
                        for r in range(2):
                            mt = 2 * p + r
                            pt = psum_pools[r].tile([128, _CHUNK], f32,
                                                    name=f"pt{r}")
                            rb = 64 * r if _STREAMS == 2 else 0
                            lhsT = yh_s[rb:rb + _K,
                                        mt * 128:(mt + 1) * 128]
                            for j in range(_CHUNK // 512):
                                n0 = c * _CHUNK + j * 512
                                nc.tensor.matmul(
                                    pt[:, j * 512:(j + 1) * 512],
                                    lhsT=lhsT,
                                    rhs=xh_s[rb:rb + _K, n0:n0 + 512],
                                    start=True, stop=True,
                                )
                            dst = slabs[r][:, c * _CHUNK:(c + 1) * _CHUNK]
                            if (mt, c) in _DVE_CASTS:
                                nc.vector.tensor_copy(dst, pt[:, :])
                            else:
                                nc.scalar.copy(out=dst, in_=pt[:, :])

                    for r in range(2):
                        mt = 2 * p + r
                        slab = slabs[r]
                        # direction A for this tile in one fused op:
                        # pairwise max of the halves + free-axis max reduce
                        if _DIRA == "ttr":
                            sc = ttrout.tile([128, _CHUNK], bf16, name="ttr")
                            nc.vector.tensor_tensor_reduce(
                                out=sc[:, :],
                                in0=slab[:, 0:_CHUNK],
                                in1=slab[:, _CHUNK:_N],
                                scale=1.0,
                                scalar=-1.0e30,
                                op0=MAX,
                                op1=MAX,
                                accum_out=macc[:, mt:mt + 1],
                            )
                        else:
                            nc.vector.tensor_reduce(
                                out=macc[:, mt:mt + 1], in_=slab[:, :],
                                op=MAX, axis=mybir.AxisListType.X)
                        # direction B: two independent running-max chains
                        if mt in _GP_TILES:
                            if first_gp[0]:
                                nc.gpsimd.tensor_copy(accG[:, :], slab[:, :])
                                first_gp[0] = False
                            else:
                                nc.gpsimd.tensor_tensor(
                                    out=accG[:, :], in0=accG[:, :],
                                    in1=slab[:, :], op=MAX)
                        elif len(first_dve) < 2 and not first_dve:
                            first_dve.append(slab)
                        elif len(first_dve) == 1:
                            nc.vector.tensor_tensor(
                                out=accD[:, :], in0=first_dve[0][:, :],
                                in1=slab[:, :], op=MAX)
                            first_dve.append(None)
                        else:
                            nc.vector.tensor_tensor(
                                out=accD[:, :], in0=accD[:, :],
                                in1=slab[:, :], op=MAX)

            # ---- merge chains + epilogue DMA (split for overlap) ----
            if _GP_TILES:
                nc.vector.tensor_tensor(
                    out=accD[:, 0:_CHUNK], in0=accD[:, 0:_CHUNK],
                    in1=accG[:, 0:_CHUNK], op=MAX)
            nc.sync.dma_start(out=outb_d.ap()[:, 0:1024],
                              in_=accD[:, 0:1024])
            nc.gpsimd.dma_start(out=outb_d.ap()[:, 1024:2048],
                                in_=accD[:, 1024:2048])
            if _GP_TILES:
                nc.vector.tensor_tensor(
                    out=accD[:, _CHUNK:_N], in0=accD[:, _CHUNK:_N],
                    in1=accG[:, _CHUNK:_N], op=MAX)
            nc.scalar.dma_start(out=outb_d.ap()[:, 2048:3072],
                                in_=accD[:, 2048:3072])
            nc.sync.dma_start(out=outb_d.ap()[:, 3072:_N],
                              in_=accD[:, 3072:_N])
            nc.scalar.dma_start(out=outm_d.ap(), in_=macc[:, :])

    nc.compile()
    return nc


def _get_program():
    global _PROGRAM
    if _PROGRAM is None:
        _PROGRAM = _build_program()
    return _PROGRAM


def _split3(a):
    """fp32 array -> 3-level bf16 split (h1 + h2 + h3 ~ a to ~2^-26 rel)."""
    h1 = a.astype(_BF16)
    r1 = a - h1.astype(np.float32)
    h2 = r1.astype(_BF16)
    r2 = r1 - h2.astype(np.float32)
    h3 = r2.astype(_BF16)
    return h1, h2, h3


def _augment(x, y):
    """x, y: (4096, 3) f32 -> xh, yh (30, 4096) bf16 such that
    sum_k yh[k, m] * xh[k, n] == -|y[m] - x[n]|^2 to ~1e-6 abs.

    Every fp32 operand is split into 3 bf16 levels; all product pairs down
    to the 2^-24 level are kept, so each product is exact in the PE's fp32
    PSUM accumulation.  Large-magnitude rows (y_sq, x_sq, hi*hi cross
    terms) come first so the running PSUM partial cancels down to ~d2
    early, keeping sequential-accumulation rounding at the fp32 noise
    floor of the reference itself."""
    xt = np.ascontiguousarray(x.T.astype(np.float32))            # (3, N)
    y2t = np.ascontiguousarray((-2.0 * y).T.astype(np.float32))  # (3, N)
    xsq = np.einsum("nd,nd->n", x, x).astype(np.float32)         # (N,)
    ysq = np.einsum("nd,nd->n", y, y).astype(np.float32)

    g1, g2, g3 = _split3(xt)
    h1, h2, h3 = _split3(y2t)
    xs1, xs2, xs3 = _split3(xsq)
    ys1, ys2, ys3 = _split3(ysq)
    ones = np.ones(_N, dtype=_BF16)

    xrows, yrows = [], []

    def add(xr, yr):
        xrows.append(xr)
        yrows.append(yr)

    add(ones, ys1)
    add(xs1, ones)
    for d in range(3):
        add(g1[d], h1[d])
    add(ones, ys2)
    add(ones, ys3)
    add(xs2, ones)
    add(xs3, ones)
    for d in range(3):
        add(g2[d], h1[d])
        add(g1[d], h2[d])
        add(g3[d], h1[d])
        add(g2[d], h2[d])
        add(g1[d], h3[d])
        add(g3[d], h2[d])
        add(g2[d], h3[d])
    xh = np.stack(xrows).astype(_BF16)
    # negate the y side so the PE emits -d2 (mins become maxes on-device)
    yh = (-np.stack(yrows).astype(np.float32)).astype(_BF16)
    assert xh.shape == (_K, _N)
    return xh, yh


def kernel(x1, y1):
    from concourse.bass_utils import run_bass_kernel_spmd

    x1 = np.asarray(x1)
    y1 = np.asarray(y1)
    assert x1.shape == (_B, _N, 3) and y1.shape == (_B, _N, 3)

    nc = _get_program()
    in_maps = []
    for b in range(_B):
        xh, yh = _augment(x1[b], y1[b])
        in_maps.append({"xh": xh, "yh": yh})

    res = run_bass_kernel_spmd(nc, in_maps, list(range(_NCORES)))
    total = 0.0
    for c in range(_NCORES):
        ma = res.results[c]["outm"].astype(np.float32)  # (128, 32): -d2min per m
        mb = res.results[c]["outb"].astype(np.float32)  # (128, 4096)
        a = ma.T.reshape(-1)                            # m = mt*128 + p
        b = mb.max(axis=0)                              # -d2min per n
        dist_a = np.sqrt(1.0e-8 + np.maximum(-a, 0.0), dtype=np.float32)
        dist_b = np.sqrt(1.0e-8 + np.maximum(-b, 0.0), dtype=np.float32)
        total += float(dist_a.sum(dtype=np.float64))
        total += float(dist_b.sum(dtype=np.float64))
    return np.float32(total / (_B * _N))


# revision 11
# speedup vs baseline: 1.3476x; 1.3476x over previous
"""Chamfer distance on 8 Trainium2 NeuronCores.

Problem: x1 (8, 4096, 3) f32, y1 (8, 4096, 3) f32.
  d2[b,m,n] = |y[b,m] - x[b,n]|^2
  out = mean_{b,n}(min_m sqrt(1e-8 + max(d2,0))) + mean_{b,m}(min_n ...)

Strategy (data-parallel over B, one batch element per core):
  * sqrt / +eps / max(.,0) are monotonic -> compute mins over raw d2 and
    apply them only to the reduced 4096-vectors on the host.
  * -d2 is produced in PSUM by matmuls with augmented K=30 inputs
    (3-level bf16 split of each fp32 operand, ~2^-26 accurate); the y
    side is negated so all on-device mins become maxes (MAX8 usable).
  * the PE runs TWO row-tiled streams (tile rows 0 and 2 of the 32x128
    tiling grid, operands replicated at SBUF partition bases 0 and 64),
    so the two weight/ifmap streams overlap and LDWEIGHTS hides.
  * per m-tile-PAIR the two streams fill one [128, 2, 4096] bf16 slab
    (ScalarE casts the four 2048-col PSUM chunks; this ~1.9us/chunk
    evacuation is pinned to ScalarE to keep the DVE free).
  * the DVE is the bottleneck (~100% busy): it runs, per pair,
      - direction B (min over m per n): 2 running-max tensor_tensors
        into a [128, 4096] accumulator (bf16 2x mode),
      - direction A (min over n per m): a halving max tree BATCHED over
        the pair via 3-D access patterns (halves per level in one op),
        finishing with MAX8 per tile into m8all.
  * outputs: m8all [128, 32*8] bf16 (host takes max of each 8) and the
    dirB accumulator [128, 4096] bf16 (host takes max over partitions).
    Output DMA is split across queues to shorten the tail.
"""

import os
import sys

for _p in ("/opt/trn_rl_repo", "/root/.axon_site/_ro/trn_rl_repo"):
    if os.path.isdir(_p) and _p not in sys.path:
        sys.path.insert(0, _p)
        break

import numpy as np
import ml_dtypes

_B = 8
_N = 4096          # points per cloud (both x and y)
_K = 30            # augmented contraction dim (3-level bf16 split)
_NCORES = 8
_MT = _N // 128    # 32 m-tiles
_CHUNK = 2048      # PSUM chunk (4 banks); 2 chunks per m-tile

_BF16 = ml_dtypes.bfloat16

# knobs
_STREAMS = int(os.environ.get("CH_STREAMS", "2"))   # 1 or 2 PE tile rows
# "mt:c" chunks cast by the DVE instead of ScalarE (DVE idles in the ramp)
_DVE_CASTS = frozenset(
    tuple(int(v) for v in t.split(":"))
    for t in os.environ.get("CH_DVE_CASTS", "0:0").split(",") if t)

_PROGRAM = None


def _build_program():
    import concourse.bacc as bacc
    import concourse.tile as tile
    import concourse.mybir as mybir

    f32 = mybir.dt.float32
    bf16 = mybir.dt.bfloat16
    MAX = mybir.AluOpType.max

    nc = bacc.Bacc("TRN2", target_bir_lowering=False, debug=False,
                   num_devices=_NCORES)

    xh_d = nc.dram_tensor("xh", [_K, _N], bf16, kind="ExternalInput")
    yh_d = nc.dram_tensor("yh", [_K, _N], bf16, kind="ExternalInput")
    outa_d = nc.dram_tensor("outa", [128, _MT * 8], bf16,
                            kind="ExternalOutput")
    outb_d = nc.dram_tensor("outb", [128, _N], bf16, kind="ExternalOutput")

    with tile.TileContext(nc) as tc:
        with tc.tile_pool(name="singles", bufs=1) as singles:
            # operand replicas at partition bases 0 and 64 (PE tile rows
            # 0 and 2 of the 32x128 row-tiling grid)
            xh_s = singles.tile([128, _N], bf16)
            yh_s = singles.tile([128, _N], bf16)
            m8all = singles.tile([128, _MT * 8], bf16)
            accB = singles.tile([128, _N], bf16)

            # input DMAs on the 3 DMA-capable queues; leading pieces
            # sized so each arrives just before the pipeline needs it
            bases = (0, 64) if _STREAMS == 2 else (0,)
            qmap = {0: nc.sync, 64: nc.scalar}
            for base in bases:
                qa = qmap[base]
                qa.dma_start(out=yh_s[base:base + _K, 0:256],
                             in_=yh_d.ap()[:, 0:256])
                qa.dma_start(out=xh_s[base:base + _K, 0:512],
                             in_=xh_d.ap()[:, 0:512])
                qa.dma_start(out=xh_s[base:base + _K, 512:2048],
                             in_=xh_d.ap()[:, 512:2048])
            for base in bases:
                nc.gpsimd.dma_start(out=xh_s[base:base + _K, 2048:_N],
                                    in_=xh_d.ap()[:, 2048:_N])
            for base in bases:
                qa = qmap[base]
                qa.dma_start(out=yh_s[base:base + _K, 256:2048],
                             in_=yh_d.ap()[:, 256:2048])
                qa.dma_start(out=yh_s[base:base + _K, 2048:_N],
                             in_=yh_d.ap()[:, 2048:_N])

            with tc.tile_pool(name="psum0", bufs=1, space="PSUM") as psum0, \
                 tc.tile_pool(name="psum1", bufs=1, space="PSUM") as psum1, \
                 tc.tile_pool(name="castp", bufs=3) as castp, \
                 tc.tile_pool(name="treep", bufs=2) as treep:
                psum_pools = (psum0, psum1)

                for p in range(_MT // 2):
                    # [partition, tile-of-pair, n] slab shared by the pair
                    slab = castp.tile([128, 2, _N], bf16, name="slab")
                    # interleave the two streams chunk-by-chunk so each
                    # stream's PSUM drain hides under the other's matmuls
                    for c in range(2):
                        for r in range(2):
                            mt = 2 * p + r
                            pt = psum_pools[r].tile([128, _CHUNK], f32,
                                                    name=f"pt{r}")
                            rb = 64 * r if _STREAMS == 2 else 0
                            lhsT = yh_s[rb:rb + _K, mt * 128:(mt + 1) * 128]
                            for j in range(_CHUNK // 512):
                                n0 = c * _CHUNK + j * 512
                                nc.tensor.matmul(
                                    pt[:, j * 512:(j + 1) * 512],
                                    lhsT=lhsT,
                                    rhs=xh_s[rb:rb + _K, n0:n0 + 512],
                                    start=True, stop=True,
                                )
                            dst = slab[:, r, c * _CHUNK:(c + 1) * _CHUNK]
                            if (mt, c) in _DVE_CASTS:
                                nc.vector.tensor_copy(dst, pt[:, :])
                            else:
                                nc.scalar.copy(out=dst, in_=pt[:, :])

                    # direction B: running max chain (first pair seeds it)
                    if p == 0:
                        nc.vector.tensor_tensor(
                            out=accB[:, :], in0=slab[:, 0, :],
                            in1=slab[:, 1, :], op=MAX)
                    else:
                        nc.vector.tensor_tensor(
                            out=accB[:, :], in0=accB[:, :],
                            in1=slab[:, 0, :], op=MAX)
                        nc.vector.tensor_tensor(
                            out=accB[:, :], in0=accB[:, :],
                            in1=slab[:, 1, :], op=MAX)

                    # direction A: halving max tree batched over the pair
                    h1 = treep.tile([128, 2, 2048], bf16, name="h1")
                    nc.vector.tensor_tensor(
                        out=h1[:, :, :], in0=slab[:, :, 0:2048],
                        in1=slab[:, :, 2048:4096], op=MAX)
                    h2 = treep.tile([128, 2, 1024], bf16, name="h2")
                    nc.vector.tensor_tensor(
                        out=h2[:, :, :], in0=h1[:, :, 0:1024],
                        in1=h1[:, :, 1024:2048], op=MAX)
                    h3 = treep.tile([128, 2, 512], bf16, name="h3")
                    nc.vector.tensor_tensor(
                        out=h3[:, :, :], in0=h2[:, :, 0:512],
                        in1=h2[:, :, 512:1024], op=MAX)
                    h4 = treep.tile([128, 2, 256], bf16, name="h4")
                    nc.vector.tensor_tensor(
                        out=h4[:, :, :], in0=h3[:, :, 0:256],
                        in1=h3[:, :, 256:512], op=MAX)
                    for r in range(2):
                        mt = 2 * p + r
                        nc.vector.max(m8all[:, mt * 8:(mt + 1) * 8],
                                      h4[:, r, :])

            # ---- epilogue DMA (split across queues to hide the tail) ----
            nc.scalar.dma_start(out=outa_d.ap(), in_=m8all[:, :])
            nc.sync.dma_start(out=outb_d.ap()[:, 0:1024],
                              in_=accB[:, 0:1024])
            nc.gpsimd.dma_start(out=outb_d.ap()[:, 1024:2048],
                                in_=accB[:, 1024:2048])
            nc.scalar.dma_start(out=outb_d.ap()[:, 2048:3072],
                                in_=accB[:, 2048:3072])
            nc.sync.dma_start(out=outb_d.ap()[:, 3072:_N],
                              in_=accB[:, 3072:_N])

    nc.compile()
    return nc


def _get_program():
    global _PROGRAM
    if _PROGRAM is None:
        _PROGRAM = _build_program()
    return _PROGRAM


def _split3(a):
    """fp32 array -> 3-level bf16 split (h1 + h2 + h3 ~ a to ~2^-26 rel)."""
    h1 = a.astype(_BF16)
    r1 = a - h1.astype(np.float32)
    h2 = r1.astype(_BF16)
    r2 = r1 - h2.astype(np.float32)
    h3 = r2.astype(_BF16)
    return h1, h2, h3


def _augment(x, y):
    """x, y: (4096, 3) f32 -> xh, yh (30, 4096) bf16 such that
    sum_k yh[k, m] * xh[k, n] == -|y[m] - x[n]|^2 to ~1e-6 abs.

    Every fp32 operand is split into 3 bf16 levels; all product pairs down
    to the 2^-24 level are kept, so each product is exact in the PE's fp32
    PSUM accumulation.  Large-magnitude rows (y_sq, x_sq, hi*hi cross
    terms) come first so the running PSUM partial cancels down to ~d2
    early, keeping sequential-accumulation rounding at the fp32 noise
    floor of the reference itself."""
    xt = np.ascontiguousarray(x.T.astype(np.float32))            # (3, N)
    y2t = np.ascontiguousarray((-2.0 * y).T.astype(np.float32))  # (3, N)
    xsq = np.einsum("nd,nd->n", x, x).astype(np.float32)         # (N,)
    ysq = np.einsum("nd,nd->n", y, y).astype(np.float32)

    g1, g2, g3 = _split3(xt)
    h1, h2, h3 = _split3(y2t)
    xs1, xs2, xs3 = _split3(xsq)
    ys1, ys2, ys3 = _split3(ysq)
    ones = np.ones(_N, dtype=_BF16)

    xrows, yrows = [], []

    def add(xr, yr):
        xrows.append(xr)
        yrows.append(yr)

    add(ones, ys1)
    add(xs1, ones)
    for d in range(3):
        add(g1[d], h1[d])
    add(ones, ys2)
    add(ones, ys3)
    add(xs2, ones)
    add(xs3, ones)
    for d in range(3):
        add(g2[d], h1[d])
        add(g1[d], h2[d])
        add(g3[d], h1[d])
        add(g2[d], h2[d])
        add(g1[d], h3[d])
        add(g3[d], h2[d])
        add(g2[d], h3[d])
    xh = np.stack(xrows).astype(_BF16)
    # negate the y side so the PE emits -d2 (mins become maxes on-device)
    yh = (-np.stack(yrows).astype(np.float32)).astype(_BF16)
    assert xh.shape == (_K, _N)
    return xh, yh


def kernel(x1, y1):
    from concourse.bass_utils import run_bass_kernel_spmd

    x1 = np.asarray(x1)
    y1 = np.asarray(y1)
    assert x1.shape == (_B, _N, 3) and y1.shape == (_B, _N, 3)

    nc = _get_program()
    in_maps = []
    for b in range(_B):
        xh, yh = _augment(x1[b], y1[b])
        in_maps.append({"xh": xh, "yh": yh})

    res = run_bass_kernel_spmd(nc, in_maps, list(range(_NCORES)))
    total = 0.0
    for c in range(_NCORES):
        ma = res.results[c]["outa"].astype(np.float32)  # (128, 32*8)
        mb = res.results[c]["outb"].astype(np.float32)  # (128, 4096)
        a = ma.reshape(128, _MT, 8).max(axis=2)         # -d2min per m
        b = mb.max(axis=0)                              # -d2min per n
        dist_a = np.sqrt(1.0e-8 + np.maximum(-a, 0.0), dtype=np.float32)
        dist_b = np.sqrt(1.0e-8 + np.maximum(-b, 0.0), dtype=np.float32)
        total += float(dist_a.sum(dtype=np.float64))
        total += float(dist_b.sum(dtype=np.float64))
    return np.float32(total / (_B * _N))
